# revision 6
# baseline (speedup 1.0000x reference)
"""Trainium2 Bass kernel for nn_ContrastiveLoss (stacked cross-attention t2i).

Strategy (8 NeuronCores, caption-sharded):
  - Each core owns 16 of the 128 captions and all 128 images.
  - Per batch of 3 images x 16 captions, compute A = im @ s^T via PE (f32r),
    the two softmaxes (word softmax normalized; region softmax's normalizer
    cancels inside cosine similarity, so only exp(9*a1) is needed), the
    cosine numerator/denominator via PE column sums, and stage per-word
    results into a [128, 800] tile.
  - One finalization pass turns staged tiles into the [128, 16] score block.
  - AllGather score blocks -> every core holds scores [128, 128]; the hinge
    margin loss (max violation) is computed on-device; host reads the scalar.

Math note: with E2 = exp(lam * a1) (unnormalized region attention),
  cos = (sum_r E2*A) / (cap_n * sqrt(E2^T G E2)) exactly, because the region
softmax normalizer cancels between numerator and |weighted context|.
"""

import numpy as np

import concourse.bass as bass
import concourse.tile as tile
from concourse import mybir
from concourse.bass_utils import run_bass_kernel_spmd
from concourse.vector_clock import ScopedClock

# ---------------------------------------------------------------------------
# Workaround for this toolchain: walrus rejects instructions carrying more
# than one semaphore wait.  Split extra waits onto standalone EventSemaphore
# instructions (the same thing wait_ge emits) just before the offender.
# ---------------------------------------------------------------------------
_PATCHED = False


def _install_patches():
    global _PATCHED
    if _PATCHED:
        return
    _PATCHED = True

    def _drain_and_barrier(self, tick_clock, wait_clock):
        nc = self.nc
        drain_inst = nc.sync.drain()
        wait_clock.add_sem_waits(
            drain_inst.ins, ScopedClock({None: tick_clock.global_clock})
        )
        waits = list(drain_inst.ins.sync_info.on_wait)
        if len(waits) > 1:
            drain_inst.ins.sync_info.on_wait = waits[:1]
            for w in waits[1:]:
                extra = nc.sync.drain()
                extra.ins.sync_info = mybir.SyncInfo(on_wait=[w], on_update=[])
        nc.all_engine_barrier()
        popped = nc._tile_sem_poison_stack.pop()
        assert popped is self._sem_poison
        nc.clear_and_free_semaphores(list(self.sems.allocated().values()))
        nc.all_engine_barrier()

    tile.TileContext._drain_and_barrier = _drain_and_barrier

    import concourse.bass_utils as bass_utils
    import concourse.bass2jax as bass2jax
    import orjson

    _orig_compile = bass_utils.compile_bir_kernel

    def _split_waits_in_bir(bir_json: bytes) -> bytes:
        m = orjson.loads(bir_json)
        for fn in m.get("functions", []):
            for blk in fn.get("blocks", []):
                insts = blk.get("instructions", [])
                new_insts = []
                for ins in insts:
                    si = ins.get("sync_info")
                    waits = (si or {}).get("on_wait") or []
                    if len(waits) > 1:
                        for k, w in enumerate(waits[:-1]):
                            new_insts.append(
                                {
                                    "name": f"{ins['name']}_wsplit{k}",
                                    "opcode": "EventSemaphore",
                                    "engine": ins["engine"],
                                    "ins": [],
                                    "outs": [],
                                    "debug": ins.get("debug"),
                                    "sync_info": {"on_update": [], "on_wait": [w]},
                                }
                            )
                        si["on_wait"] = waits[-1:]
                    new_insts.append(ins)
                blk["instructions"] = new_insts
        return orjson.dumps(m)

    def _patched_compile(bir_json, tmpdir, neff_name="file.neff"):
        return _orig_compile(_split_waits_in_bir(bir_json), tmpdir, neff_name)

    bass_utils.compile_bir_kernel = _patched_compile
    bass2jax.compile_bir_kernel = _patched_compile


# ---------------------------------------------------------------------------
# Problem constants (hardcoded per the task contract).
# ---------------------------------------------------------------------------
B = 128           # images == captions
LI = 36           # image regions
LW = 50           # padded caption words
D = 1024          # feature dim
NC = 8            # cores
CAP = B // NC     # captions per core (16)
WF = CAP * LW     # free width of the batched tiles (800)
IMG_GRP = 3       # images per batch
NB = (B + IMG_GRP - 1) // IMG_GRP  # 43 batches (42x3 + 1x2)
LAM = 9.0
MARGIN = 0.2
EPS = 1e-8
MASKNEG = -30000.0

F32 = mybir.dt.float32
F32R = mybir.dt.float32r

# When True, subtract a per-(row, caption)-segment max before the word
# softmax (exactly like the reference).  When False, use a per-row global max
# as the exp offset (one fewer pass; softmax value is identical unless an
# entire segment underflows).
SEGMAX = True

_CACHE = {}


def _build_program():
    nc = bass.Bass("TRN2", target_bir_lowering=False, debug=False, num_devices=NC)

    # Inputs (per-core contents differ only for sT8 / mask / wfac).
    imT8 = nc.dram_tensor("imT8", [8, 128, B * LI], F32R, kind="ExternalInput")
    sT8 = nc.dram_tensor("sT8", [8, 128, WF], F32R, kind="ExternalInput")
    g43 = nc.dram_tensor("g43", [NB, IMG_GRP * LI, IMG_GRP * LI], F32R, kind="ExternalInput")
    maskneg_d = nc.dram_tensor("maskneg", [1, WF], F32R, kind="ExternalInput")
    wfac_d = nc.dram_tensor("wfac", [128, WF], F32, kind="ExternalInput")
    eye_d = nc.dram_tensor("eye", [128, 128], F32, kind="ExternalInput")
    noteye_d = nc.dram_tensor("noteye", [128, 128], F32, kind="ExternalInput")
    onesblk_d = nc.dram_tensor("onesblk", [IMG_GRP * LI, IMG_GRP], F32R, kind="ExternalInput")
    ones1_d = nc.dram_tensor("ones1", [1, IMG_GRP * LI], F32R, kind="ExternalInput")
    ones128_d = nc.dram_tensor("ones128", [128, 1], F32R, kind="ExternalInput")

    loss_out = nc.dram_tensor("loss_out", [1, 2], F32, kind="ExternalOutput")
    scores_out = nc.dram_tensor("scores_out", [128, 128], F32, kind="ExternalOutput")

    with tile.TileContext(nc) as tc:
        with (
            tc.tile_pool(name="const", bufs=1) as cpool,
            tc.tile_pool(name="imp", bufs=3) as impool,
            tc.tile_pool(name="gp", bufs=2) as gpool,
            tc.tile_pool(name="work", bufs=2) as work,
            tc.tile_pool(name="small", bufs=2) as small,
            tc.tile_pool(name="stage", bufs=1) as stage,
            tc.tile_pool(name="pa", bufs=2, space="PSUM") as pa,
            tc.tile_pool(name="pc", bufs=2, space="PSUM") as pc,
            tc.tile_pool(name="dram", bufs=1, space="DRAM") as dram,
        ):
            # ---- persistent tiles -------------------------------------------------
            sT = cpool.tile([128, 8, WF], F32R, tag="sT")
            nc.sync.dma_start(sT[:], sT8[:].transpose([1, 0, 2]))
            masknegt = cpool.tile([1, WF], F32R, tag="mn")
            nc.sync.dma_start(masknegt[:], maskneg_d[:])
            wfact = cpool.tile([128, WF], F32, tag="wf")
            nc.sync.dma_start(wfact[:], wfac_d[:])
            eyet = cpool.tile([128, 128], F32, tag="eye")
            nc.sync.dma_start(eyet[:], eye_d[:])
            noteyet = cpool.tile([128, 128], F32, tag="neye")
            nc.sync.dma_start(noteyet[:], noteye_d[:])
            onesblkt = cpool.tile([IMG_GRP * LI, IMG_GRP], F32R, tag="ob")
            nc.sync.dma_start(onesblkt[:], onesblk_d[:])
            ones1t = cpool.tile([1, IMG_GRP * LI], F32R, tag="o1")
            nc.sync.dma_start(ones1t[:], ones1_d[:])
            ones128t = cpool.tile([128, 1], F32R, tag="o128")
            nc.sync.dma_start(ones128t[:], ones128_d[:])

            nst = stage.tile([128, WF], F32, tag="nst")
            wst = stage.tile([128, WF], F32, tag="wst")

            NCH = [(0, 512), (512, WF)]

            # ---- main loop over image groups -------------------------------------
            for b in range(NB):
                ng = min(IMG_GRP, B - b * IMG_GRP)   # images in this group
                P = ng * LI                          # partitions used

                imb = impool.tile([128, 8, P], F32R, tag="imb")
                nc.sync.dma_start(
                    imb[:], imT8[:, :, b * IMG_GRP * LI : b * IMG_GRP * LI + P].transpose([1, 0, 2])
                )
                gt = gpool.tile([P, P], F32R, tag="gt")
                nc.sync.dma_start(gt[:], g43[b, 0:P, 0:P])

                # A[P, WF] = sum_c imb_c^T @ sT_c  (+ word mask row)
                a_ps = pa.tile([P, WF], F32, tag="AT")
                for n0, n1 in NCH:
                    for c in range(8):
                        nc.tensor.matmul(
                            a_ps[:, n0:n1], imb[:, c, :], sT[:, c, n0:n1],
                            start=(c == 0), stop=False,
                        )
                    nc.tensor.matmul(
                        a_ps[:, n0:n1], ones1t[0:1, 0:P], masknegt[0:1, n0:n1],
                        start=False, stop=True,
                    )

                am = work.tile([P, WF], F32, tag="am")
                nc.scalar.copy(am[:], a_ps[:])
                e = work.tile([P, WF], F32, tag="e")
                if SEGMAX:
                    mx = small.tile([P, CAP], F32, tag="mx")
                    nc.vector.tensor_reduce(
                        mx[:], a_ps[:].rearrange("p (c w) -> p c w", c=CAP, w=LW),
                        axis=mybir.AxisListType.X, op=mybir.AluOpType.max,
                    )
                    sub = work.tile([P, WF], F32, tag="sub")
                    nc.gpsimd.tensor_tensor(
                        sub[:].rearrange("p (c w) -> p c w", c=CAP, w=LW),
                        am[:].rearrange("p (c w) -> p c w", c=CAP, w=LW),
                        mx[:].unsqueeze(2).broadcast_to([P, CAP, LW]),
                        op=mybir.AluOpType.subtract,
                    )
                    nc.scalar.activation(e[:], sub[:], mybir.ActivationFunctionType.Exp)
                else:
                    negmax = small.tile([P, 1], F32, tag="negmax")
                    nc.vector.tensor_reduce(
                        negmax[:], a_ps[:], axis=mybir.AxisListType.X,
                        op=mybir.AluOpType.max, negate=True,
                    )
                    nc.scalar.activation(
                        e[:], a_ps[:], mybir.ActivationFunctionType.Exp,
                        bias=negmax[:], scale=1.0,
                    )

                z = small.tile([P, CAP], F32, tag="z")
                nc.vector.tensor_reduce(
                    z[:], e[:].rearrange("p (c w) -> p c w", c=CAP, w=LW),
                    axis=mybir.AxisListType.X, op=mybir.AluOpType.add,
                )
                rz = small.tile([P, CAP], F32, tag="rz")
                nc.vector.reciprocal(rz[:], z[:])

                m = work.tile([P, WF], F32, tag="m")
                nc.vector.tensor_tensor(
                    m[:].rearrange("p (c w) -> p c w", c=CAP, w=LW),
                    e[:].rearrange("p (c w) -> p c w", c=CAP, w=LW),
                    rz[:].unsqueeze(2).broadcast_to([P, CAP, LW]),
                    op=mybir.AluOpType.mult,
                )
                e2 = work.tile([P, WF], F32R, tag="e2")
                nc.scalar.activation(
                    e2[:], m[:], mybir.ActivationFunctionType.Exp, bias=0.0, scale=LAM
                )

                f = work.tile([P, WF], F32R, tag="f")
                nc.gpsimd.tensor_tensor(f[:], am[:], e2[:], op=mybir.AluOpType.mult)

                t_ps = pa.tile([P, WF], F32, tag="AT")
                for n0, n1 in NCH:
                    nc.tensor.matmul(t_ps[:, n0:n1], gt[:], e2[:, n0:n1], start=True, stop=True)

                u = work.tile([P, WF], F32R, tag="u")
                nc.vector.tensor_tensor(u[:], t_ps[:], e2[:], op=mybir.AluOpType.mult)

                n_ps = pc.tile([ng, WF], F32, tag="cs")
                for n0, n1 in NCH:
                    nc.tensor.matmul(n_ps[:, n0:n1], onesblkt[0:P, 0:ng], f[:, n0:n1], start=True, stop=True)
                w_ps = pc.tile([ng, WF], F32, tag="cs")
                for n0, n1 in NCH:
                    nc.tensor.matmul(w_ps[:, n0:n1], onesblkt[0:P, 0:ng], u[:, n0:n1], start=True, stop=True)

                r0 = b * IMG_GRP
                nb_sb = small.tile([ng, WF], F32, tag="nb_sb")
                wb_sb = small.tile([ng, WF], F32, tag="wb_sb")
                nc.scalar.copy(nb_sb[:], n_ps[:])
                nc.scalar.copy(wb_sb[:], w_ps[:])
                nc.sync.dma_start(nst[r0 : r0 + ng, :], nb_sb[:])
                nc.sync.dma_start(wst[r0 : r0 + ng, :], wb_sb[:])

            # ---- finalize: scores block [128 images, 16 captions] ----------------
            srt = work.tile([128, WF], F32, tag="am")
            nc.scalar.sqrt(srt[:], wst[:])
            q = work.tile([128, WF], F32, tag="e")
            nc.vector.tensor_tensor(q[:], nst[:], wfact[:], op=mybir.AluOpType.mult)
            rsq = work.tile([128, WF], F32, tag="sub" if SEGMAX else "f")
            nc.vector.reciprocal(rsq[:], srt[:])
            cosq = work.tile([128, WF], F32, tag="m")
            nc.vector.tensor_tensor(cosq[:], q[:], rsq[:], op=mybir.AluOpType.mult)
            sim = small.tile([128, CAP], F32, tag="sim")
            nc.vector.tensor_reduce(
                sim[:], cosq[:].rearrange("p (c w) -> p c w", c=CAP, w=LW),
                axis=mybir.AxisListType.X, op=mybir.AluOpType.add,
            )

            # ---- all-gather the score columns ------------------------------------
            ag_in = dram.tile([128, CAP], F32)
            ag_out = dram.tile([NC, 128, CAP], F32, addr_space="Shared")
            nc.sync.dma_start(ag_in[:], sim[:])
            nc.gpsimd.collective_compute(
                "AllGather",
                mybir.AluOpType.bypass,
                replica_groups=[list(range(NC))],
                ins=[ag_in.opt()],
                outs=[ag_out.opt()],
            )
            s_t = cpool.tile([128, NC, CAP], F32, tag="scores")
            nc.sync.dma_start(s_t[:], ag_out[:].transpose([1, 0, 2]))
            s2d = s_t[:].rearrange("p c w -> p (c w)")
            nc.sync.dma_start(scores_out[:], s2d)

            # ---- margin loss (every core computes it; core 0's is read) ----------
            junk = work.tile([128, 128], F32, tag="am")
            diag = small.tile([128, 1], F32, tag="diag")
            nc.vector.tensor_tensor(junk[:, 0:128], s2d, eyet[:], op=mybir.AluOpType.mult)
            nc.vector.tensor_reduce(
                diag[:], junk[:, 0:128], axis=mybir.AxisListType.X, op=mybir.AluOpType.add
            )
            bias = small.tile([128, 1], F32, tag="bias")
            nc.vector.tensor_scalar(
                bias[:], diag[:], scalar1=-1.0, scalar2=MARGIN,
                op0=mybir.AluOpType.mult, op1=mybir.AluOpType.add,
            )
            # cost_s = relu(S + margin - d_i), diagonal zeroed
            cs = work.tile([128, 128], F32, tag="e")
            nc.scalar.activation(
                cs[:], s2d, mybir.ActivationFunctionType.Relu, bias=bias[:], scale=1.0
            )
            cs2 = work.tile([128, 128], F32, tag="m")
            nc.vector.tensor_tensor(cs2[:], cs[:], noteyet[:], op=mybir.AluOpType.mult)
            rmaxs = small.tile([128, 2], F32R, tag="rmaxs")
            nc.vector.tensor_reduce(
                rmaxs[:, 0:1], cs2[:], axis=mybir.AxisListType.X, op=mybir.AluOpType.max
            )
            # transposed scores for cost_im
            st_ps = pc.tile([128, 128], F32, tag="cs")
            nc.tensor.transpose(st_ps[:], s_t[:].rearrange("p c w -> p (c w)"), eyet[:])
            ct = work.tile([128, 128], F32, tag="u")
            nc.scalar.activation(
                ct[:], st_ps[:], mybir.ActivationFunctionType.Relu, bias=bias[:], scale=1.0
            )
            ct2 = work.tile([128, 128], F32, tag="f")
            nc.vector.tensor_tensor(ct2[:], ct[:], noteyet[:], op=mybir.AluOpType.mult)
            nc.vector.tensor_reduce(
                rmaxs[:, 1:2], ct2[:], axis=mybir.AxisListType.X, op=mybir.AluOpType.max
            )
            tot_ps = pc.tile([1, 2], F32, tag="cs")
            nc.tensor.matmul(tot_ps[:], ones128t[:], rmaxs[:], start=True, stop=True)
            tot = small.tile([1, 2], F32, tag="tot")
            nc.scalar.copy(tot[:], tot_ps[:])
            nc.sync.dma_start(loss_out[:], tot[:])

    return nc


def _host_prep(im, s, s_l):
    im = np.ascontiguousarray(im, dtype=np.float32)
    s = np.ascontiguousarray(s, dtype=np.float32)
    s_l = np.asarray(s_l).astype(np.int64)

    # imT8[c, d, i*LI+r] = im[i, r, c*128+d]
    imT = im.reshape(B * LI, D).T            # [D, B*LI]
    imT8 = np.ascontiguousarray(imT.reshape(8, 128, B * LI))

    # gram matrices, block-diagonal per image group
    G = np.matmul(im, im.transpose(0, 2, 1))  # [B, LI, LI]
    g43 = np.zeros((NB, IMG_GRP * LI, IMG_GRP * LI), dtype=np.float32)
    for b in range(NB):
        ng = min(IMG_GRP, B - b * IMG_GRP)
        for g in range(ng):
            g43[b, g * LI : (g + 1) * LI, g * LI : (g + 1) * LI] = G[b * IMG_GRP + g]

    eye = np.eye(128, dtype=np.float32)
    noteye = 1.0 - eye
    onesblk = np.zeros((IMG_GRP * LI, IMG_GRP), dtype=np.float32)
    for g in range(IMG_GRP):
        onesblk[g * LI : (g + 1) * LI, g] = 1.0
    ones1 = np.ones((1, IMG_GRP * LI), dtype=np.float32)
    ones128 = np.ones((128, 1), dtype=np.float32)

    wmask_all = (np.arange(LW)[None, :] < s_l[:, None]).astype(np.float32)  # [B, LW]
    capn_all = np.linalg.norm(s, axis=-1)                                    # [B, LW]

    in_maps = []
    for core in range(NC):
        j0 = core * CAP
        sj = s[j0 : j0 + CAP]                       # [CAP, LW, D]
        sT = sj.reshape(WF, D).T                    # [D, WF]
        sT8 = np.ascontiguousarray(sT.reshape(8, 128, WF))
        wm = wmask_all[j0 : j0 + CAP]               # [CAP, LW]
        capn = capn_all[j0 : j0 + CAP]
        maskneg = ((1.0 - wm) * MASKNEG).reshape(1, WF).astype(np.float32)
        lens = s_l[j0 : j0 + CAP].astype(np.float32)[:, None]
        wfac = (wm / (np.maximum(capn, EPS) * lens)).reshape(WF).astype(np.float32)
        wfac = np.broadcast_to(wfac, (128, WF)).copy()
        in_maps.append(
            {
                "imT8": imT8,
                "sT8": sT8,
                "g43": g43,
                "maskneg": maskneg,
                "wfac": wfac,
                "eye": eye,
                "noteye": noteye,
                "onesblk": onesblk,
                "ones1": ones1,
                "ones128": ones128,
            }
        )
    return in_maps


def run(im, s, s_l, trace=False):
    """Returns (loss_scalar, scores[128,128], bass_results)."""
    _install_patches()
    if "nc" not in _CACHE:
        _CACHE["nc"] = _build_program()
    nc = _CACHE["nc"]
    in_maps = _host_prep(im, s, s_l)
    res = run_bass_kernel_spmd(nc, in_maps, list(range(NC)), trace=trace)
    r0 = res.results[0]
    loss = np.float32(r0["loss_out"][0, 0] + r0["loss_out"][0, 1])
    return loss, r0["scores_out"], res


def kernel(im, s, s_l):
    loss, _, _ = run(im, s, s_l)
    return np.array(loss, dtype=np.float32)


# revision 7
# speedup vs baseline: 1.0372x; 1.0372x over previous
"""Trainium2 Bass kernel for nn_ContrastiveLoss (stacked cross-attention t2i).

Strategy (8 NeuronCores, caption-sharded):
  - Each core owns 16 of the 128 captions and all 128 images.
  - Per batch of 3 images x 16 captions, compute A = im @ s^T via PE (f32r),
    the two softmaxes (word softmax normalized; region softmax's normalizer
    cancels inside cosine similarity, so only exp(9*a1) is needed), the
    cosine numerator/denominator via PE column sums, and stage per-word
    results into a [128, 800] tile.
  - One finalization pass turns staged tiles into the [128, 16] score block.
  - AllGather score blocks -> every core holds scores [128, 128]; the hinge
    margin loss (max violation) is computed on-device; host reads the scalar.

Math note: with E2 = exp(lam * a1) (unnormalized region attention),
  cos = (sum_r E2*A) / (cap_n * sqrt(E2^T G E2)) exactly, because the region
softmax normalizer cancels between numerator and |weighted context|.
"""

import numpy as np

import concourse.bass as bass
import concourse.tile as tile
from concourse import mybir
from concourse.bass_utils import run_bass_kernel_spmd
from concourse.vector_clock import ScopedClock

# ---------------------------------------------------------------------------
# Workaround for this toolchain: walrus rejects instructions carrying more
# than one semaphore wait.  Split extra waits onto standalone EventSemaphore
# instructions (the same thing wait_ge emits) just before the offender.
# ---------------------------------------------------------------------------
_PATCHED = False


def _install_patches():
    global _PATCHED
    if _PATCHED:
        return
    _PATCHED = True

    def _drain_and_barrier(self, tick_clock, wait_clock):
        nc = self.nc
        drain_inst = nc.sync.drain()
        wait_clock.add_sem_waits(
            drain_inst.ins, ScopedClock({None: tick_clock.global_clock})
        )
        waits = list(drain_inst.ins.sync_info.on_wait)
        if len(waits) > 1:
            drain_inst.ins.sync_info.on_wait = waits[:1]
            for w in waits[1:]:
                extra = nc.sync.drain()
                extra.ins.sync_info = mybir.SyncInfo(on_wait=[w], on_update=[])
        nc.all_engine_barrier()
        popped = nc._tile_sem_poison_stack.pop()
        assert popped is self._sem_poison
        nc.clear_and_free_semaphores(list(self.sems.allocated().values()))
        nc.all_engine_barrier()

    tile.TileContext._drain_and_barrier = _drain_and_barrier

    import concourse.bass_utils as bass_utils
    import concourse.bass2jax as bass2jax
    import orjson

    _orig_compile = bass_utils.compile_bir_kernel

    def _split_waits_in_bir(bir_json: bytes) -> bytes:
        m = orjson.loads(bir_json)
        for fn in m.get("functions", []):
            for blk in fn.get("blocks", []):
                insts = blk.get("instructions", [])
                new_insts = []
                for ins in insts:
                    si = ins.get("sync_info")
                    waits = (si or {}).get("on_wait") or []
                    if len(waits) > 1:
                        for k, w in enumerate(waits[:-1]):
                            new_insts.append(
                                {
                                    "name": f"{ins['name']}_wsplit{k}",
                                    "opcode": "EventSemaphore",
                                    "engine": ins["engine"],
                                    "ins": [],
                                    "outs": [],
                                    "debug": ins.get("debug"),
                                    "sync_info": {"on_update": [], "on_wait": [w]},
                                }
                            )
                        si["on_wait"] = waits[-1:]
                    new_insts.append(ins)
                blk["instructions"] = new_insts
        return orjson.dumps(m)

    def _patched_compile(bir_json, tmpdir, neff_name="file.neff"):
        return _orig_compile(_split_waits_in_bir(bir_json), tmpdir, neff_name)

    bass_utils.compile_bir_kernel = _patched_compile
    bass2jax.compile_bir_kernel = _patched_compile


# ---------------------------------------------------------------------------
# Problem constants (hardcoded per the task contract).
# ---------------------------------------------------------------------------
B = 128           # images == captions
LI = 36           # image regions
LW = 50           # padded caption words
D = 1024          # feature dim
NC = 8            # cores
CAP = B // NC     # captions per core (16)
WF = CAP * LW     # free width of the batched tiles (800)
IMG_GRP = 3       # images per batch
NB = (B + IMG_GRP - 1) // IMG_GRP  # 43 batches (42x3 + 1x2)
LAM = 9.0
MARGIN = 0.2
EPS = 1e-8
MASKNEG = -30000.0

F32 = mybir.dt.float32
F32R = mybir.dt.float32r

# When True, subtract a per-(row, caption)-segment max before the word
# softmax (exactly like the reference).  When False, use a per-row global max
# as the exp offset (one fewer pass; softmax value is identical unless an
# entire segment underflows).
SEGMAX = True

_CACHE = {}


def _build_program():
    nc = bass.Bass("TRN2", target_bir_lowering=False, debug=False, num_devices=NC)

    # Inputs (per-core contents differ only for sT8 / mask / wfac).
    imT8 = nc.dram_tensor("imT8", [8, 128, B * LI], F32R, kind="ExternalInput")
    sT8 = nc.dram_tensor("sT8", [8, 128, WF], F32R, kind="ExternalInput")
    g43 = nc.dram_tensor("g43", [NB, IMG_GRP * LI, IMG_GRP * LI], F32R, kind="ExternalInput")
    maskneg_d = nc.dram_tensor("maskneg", [1, WF], F32R, kind="ExternalInput")
    wfac_d = nc.dram_tensor("wfac", [128, WF], F32, kind="ExternalInput")
    eye_d = nc.dram_tensor("eye", [128, 128], F32, kind="ExternalInput")
    noteye_d = nc.dram_tensor("noteye", [128, 128], F32, kind="ExternalInput")
    onesblk_d = nc.dram_tensor("onesblk", [IMG_GRP * LI, IMG_GRP], F32R, kind="ExternalInput")
    ones1_d = nc.dram_tensor("ones1", [1, IMG_GRP * LI], F32R, kind="ExternalInput")
    ones128_d = nc.dram_tensor("ones128", [128, 1], F32R, kind="ExternalInput")

    loss_out = nc.dram_tensor("loss_out", [1, 2], F32, kind="ExternalOutput")
    scores_out = nc.dram_tensor("scores_out", [128, 128], F32, kind="ExternalOutput")

    with tile.TileContext(nc) as tc:
        with (
            tc.tile_pool(name="const", bufs=1) as cpool,
            tc.tile_pool(name="imp", bufs=3) as impool,
            tc.tile_pool(name="gp", bufs=2) as gpool,
            tc.tile_pool(name="work", bufs=2) as work,
            tc.tile_pool(name="small", bufs=2) as small,
            tc.tile_pool(name="stage", bufs=1) as stage,
            tc.tile_pool(name="pa", bufs=2, space="PSUM") as pa,
            tc.tile_pool(name="pc", bufs=2, space="PSUM") as pc,
            tc.tile_pool(name="dram", bufs=1, space="DRAM") as dram,
        ):
            # ---- persistent tiles -------------------------------------------------
            sT = cpool.tile([128, 8, WF], F32R, tag="sT")
            nc.sync.dma_start(sT[:], sT8[:].transpose([1, 0, 2]))
            masknegt = cpool.tile([1, WF], F32R, tag="mn")
            nc.sync.dma_start(masknegt[:], maskneg_d[:])
            wfact = cpool.tile([128, WF], F32, tag="wf")
            nc.sync.dma_start(wfact[:], wfac_d[:])
            eyet = cpool.tile([128, 128], F32, tag="eye")
            nc.sync.dma_start(eyet[:], eye_d[:])
            noteyet = cpool.tile([128, 128], F32, tag="neye")
            nc.sync.dma_start(noteyet[:], noteye_d[:])
            onesblkt = cpool.tile([IMG_GRP * LI, IMG_GRP], F32R, tag="ob")
            nc.sync.dma_start(onesblkt[:], onesblk_d[:])
            ones1t = cpool.tile([1, IMG_GRP * LI], F32R, tag="o1")
            nc.sync.dma_start(ones1t[:], ones1_d[:])
            ones128t = cpool.tile([128, 1], F32R, tag="o128")
            nc.sync.dma_start(ones128t[:], ones128_d[:])

            nst = stage.tile([128, WF], F32, tag="nst")
            wst = stage.tile([128, WF], F32, tag="wst")

            NCH = [(0, 512), (512, WF)]

            # ---- main loop over image groups -------------------------------------
            for b in range(NB):
                ng = min(IMG_GRP, B - b * IMG_GRP)   # images in this group
                P = ng * LI                          # partitions used

                imb = impool.tile([128, 8, P], F32R, tag="imb")
                nc.sync.dma_start(
                    imb[:], imT8[:, :, b * IMG_GRP * LI : b * IMG_GRP * LI + P].transpose([1, 0, 2])
                )
                gt = gpool.tile([P, P], F32R, tag="gt")
                nc.sync.dma_start(gt[:], g43[b, 0:P, 0:P])

                # A[P, WF] = sum_c imb_c^T @ sT_c  (+ word mask row)
                a_ps = pa.tile([P, WF], F32, tag="AT")
                for n0, n1 in NCH:
                    for c in range(8):
                        nc.tensor.matmul(
                            a_ps[:, n0:n1], imb[:, c, :], sT[:, c, n0:n1],
                            start=(c == 0), stop=False,
                        )
                    nc.tensor.matmul(
                        a_ps[:, n0:n1], ones1t[0:1, 0:P], masknegt[0:1, n0:n1],
                        start=False, stop=True,
                    )

                am = work.tile([P, WF], F32, tag="am")
                nc.scalar.copy(am[:], a_ps[:])
                e = work.tile([P, WF], F32, tag="e")
                if SEGMAX:
                    mx = small.tile([P, CAP], F32, tag="mx")
                    nc.vector.tensor_reduce(
                        mx[:], a_ps[:].rearrange("p (c w) -> p c w", c=CAP, w=LW),
                        axis=mybir.AxisListType.X, op=mybir.AluOpType.max,
                    )
                    sub = work.tile([P, WF], F32, tag="sub")
                    nc.gpsimd.tensor_tensor(
                        sub[:].rearrange("p (c w) -> p c w", c=CAP, w=LW),
                        am[:].rearrange("p (c w) -> p c w", c=CAP, w=LW),
                        mx[:].unsqueeze(2).broadcast_to([P, CAP, LW]),
                        op=mybir.AluOpType.subtract,
                    )
                    nc.scalar.activation(e[:], sub[:], mybir.ActivationFunctionType.Exp)
                else:
                    negmax = small.tile([P, 1], F32, tag="negmax")
                    nc.vector.tensor_reduce(
                        negmax[:], a_ps[:], axis=mybir.AxisListType.X,
                        op=mybir.AluOpType.max, negate=True,
                    )
                    nc.scalar.activation(
                        e[:], a_ps[:], mybir.ActivationFunctionType.Exp,
                        bias=negmax[:], scale=1.0,
                    )

                z = small.tile([P, CAP], F32, tag="z")
                nc.vector.tensor_reduce(
                    z[:], e[:].rearrange("p (c w) -> p c w", c=CAP, w=LW),
                    axis=mybir.AxisListType.X, op=mybir.AluOpType.add,
                )
                rz = small.tile([P, CAP], F32, tag="rz")
                nc.vector.reciprocal(rz[:], z[:])

                m = work.tile([P, WF], F32, tag="m")
                nc.vector.tensor_tensor(
                    m[:].rearrange("p (c w) -> p c w", c=CAP, w=LW),
                    e[:].rearrange("p (c w) -> p c w", c=CAP, w=LW),
                    rz[:].unsqueeze(2).broadcast_to([P, CAP, LW]),
                    op=mybir.AluOpType.mult,
                )
                e2 = work.tile([P, WF], F32R, tag="e2")
                nc.scalar.activation(
                    e2[:], m[:], mybir.ActivationFunctionType.Exp, bias=0.0, scale=LAM
                )

                f = work.tile([P, WF], F32R, tag="f")
                nc.gpsimd.tensor_tensor(f[:], am[:], e2[:], op=mybir.AluOpType.mult)

                t_ps = pa.tile([P, WF], F32, tag="AT")
                for n0, n1 in NCH:
                    nc.tensor.matmul(t_ps[:, n0:n1], gt[:], e2[:, n0:n1], start=True, stop=True)

                u = work.tile([P, WF], F32R, tag="u")
                nc.vector.tensor_tensor(u[:], t_ps[:], e2[:], op=mybir.AluOpType.mult)

                n_ps = pc.tile([ng, WF], F32, tag="cs")
                for n0, n1 in NCH:
                    nc.tensor.matmul(n_ps[:, n0:n1], onesblkt[0:P, 0:ng], f[:, n0:n1], start=True, stop=True)
                w_ps = pc.tile([ng, WF], F32, tag="cs")
                for n0, n1 in NCH:
                    nc.tensor.matmul(w_ps[:, n0:n1], onesblkt[0:P, 0:ng], u[:, n0:n1], start=True, stop=True)

                r0 = b * IMG_GRP
                nb_sb = small.tile([ng, WF], F32, tag="nb_sb")
                wb_sb = small.tile([ng, WF], F32, tag="wb_sb")
                nc.scalar.copy(nb_sb[:], n_ps[:])
                nc.scalar.copy(wb_sb[:], w_ps[:])
                nc.sync.dma_start(nst[r0 : r0 + ng, :], nb_sb[:])
                nc.sync.dma_start(wst[r0 : r0 + ng, :], wb_sb[:])

            # ---- finalize: scores block [128 images, 16 captions] ----------------
            srt = work.tile([128, WF], F32, tag="am")
            nc.scalar.sqrt(srt[:], wst[:])
            q = work.tile([128, WF], F32, tag="e")
            nc.vector.tensor_tensor(q[:], nst[:], wfact[:], op=mybir.AluOpType.mult)
            rsq = work.tile([128, WF], F32, tag="sub" if SEGMAX else "f")
            nc.vector.reciprocal(rsq[:], srt[:])
            cosq = work.tile([128, WF], F32, tag="m")
            nc.vector.tensor_tensor(cosq[:], q[:], rsq[:], op=mybir.AluOpType.mult)
            sim = small.tile([128, CAP], F32, tag="sim")
            nc.vector.tensor_reduce(
                sim[:], cosq[:].rearrange("p (c w) -> p c w", c=CAP, w=LW),
                axis=mybir.AxisListType.X, op=mybir.AluOpType.add,
            )

            # ---- all-gather the score columns ------------------------------------
            ag_in = dram.tile([128, CAP], F32)
            ag_out = dram.tile([NC, 128, CAP], F32, addr_space="Shared")
            nc.sync.dma_start(ag_in[:], sim[:])
            nc.gpsimd.collective_compute(
                "AllGather",
                mybir.AluOpType.bypass,
                replica_groups=[list(range(NC))],
                ins=[ag_in.opt()],
                outs=[ag_out.opt()],
            )
            s_t = cpool.tile([128, NC, CAP], F32, tag="scores")
            nc.sync.dma_start(s_t[:], ag_out[:].transpose([1, 0, 2]))
            s2d = s_t[:].rearrange("p c w -> p (c w)")
            nc.sync.dma_start(scores_out[:], s2d)

            # ---- margin loss (every core computes it; core 0's is read) ----------
            junk = work.tile([128, 128], F32, tag="am")
            diag = small.tile([128, 1], F32, tag="diag")
            nc.vector.tensor_tensor(junk[:, 0:128], s2d, eyet[:], op=mybir.AluOpType.mult)
            nc.vector.tensor_reduce(
                diag[:], junk[:, 0:128], axis=mybir.AxisListType.X, op=mybir.AluOpType.add
            )
            bias = small.tile([128, 1], F32, tag="bias")
            nc.vector.tensor_scalar(
                bias[:], diag[:], scalar1=-1.0, scalar2=MARGIN,
                op0=mybir.AluOpType.mult, op1=mybir.AluOpType.add,
            )
            # cost_s = relu(S + margin - d_i), diagonal zeroed
            cs = work.tile([128, 128], F32, tag="e")
            nc.scalar.activation(
                cs[:], s2d, mybir.ActivationFunctionType.Relu, bias=bias[:], scale=1.0
            )
            cs2 = work.tile([128, 128], F32, tag="m")
            nc.vector.tensor_tensor(cs2[:], cs[:], noteyet[:], op=mybir.AluOpType.mult)
            rmaxs = small.tile([128, 2], F32R, tag="rmaxs")
            nc.vector.tensor_reduce(
                rmaxs[:, 0:1], cs2[:], axis=mybir.AxisListType.X, op=mybir.AluOpType.max
            )
            # transposed scores for cost_im
            st_ps = pc.tile([128, 128], F32, tag="cs")
            nc.tensor.transpose(st_ps[:], s_t[:].rearrange("p c w -> p (c w)"), eyet[:])
            ct = work.tile([128, 128], F32, tag="u")
            nc.scalar.activation(
                ct[:], st_ps[:], mybir.ActivationFunctionType.Relu, bias=bias[:], scale=1.0
            )
            ct2 = work.tile([128, 128], F32, tag="f")
            nc.vector.tensor_tensor(ct2[:], ct[:], noteyet[:], op=mybir.AluOpType.mult)
            nc.vector.tensor_reduce(
                rmaxs[:, 1:2], ct2[:], axis=mybir.AxisListType.X, op=mybir.AluOpType.max
            )
            tot_ps = pc.tile([1, 2], F32, tag="cs")
            nc.tensor.matmul(tot_ps[:], ones128t[:], rmaxs[:], start=True, stop=True)
            tot = small.tile([1, 2], F32, tag="tot")
            nc.scalar.copy(tot[:], tot_ps[:])
            nc.sync.dma_start(loss_out[:], tot[:])

    return nc


def _host_prep(im, s, s_l):
    im = np.ascontiguousarray(im, dtype=np.float32)
    s = np.ascontiguousarray(s, dtype=np.float32)
    s_l = np.asarray(s_l).astype(np.int64)

    # imT8[c, d, i*LI+r] = im[i, r, c*128+d]
    imT = im.reshape(B * LI, D).T            # [D, B*LI]
    imT8 = np.ascontiguousarray(imT.reshape(8, 128, B * LI))

    # gram matrices, block-diagonal per image group
    G = np.matmul(im, im.transpose(0, 2, 1))  # [B, LI, LI]
    g43 = np.zeros((NB, IMG_GRP * LI, IMG_GRP * LI), dtype=np.float32)
    for b in range(NB):
        ng = min(IMG_GRP, B - b * IMG_GRP)
        for g in range(ng):
            g43[b, g * LI : (g + 1) * LI, g * LI : (g + 1) * LI] = G[b * IMG_GRP + g]

    eye = np.eye(128, dtype=np.float32)
    noteye = 1.0 - eye
    onesblk = np.zeros((IMG_GRP * LI, IMG_GRP), dtype=np.float32)
    for g in range(IMG_GRP):
        onesblk[g * LI : (g + 1) * LI, g] = 1.0
    ones1 = np.ones((1, IMG_GRP * LI), dtype=np.float32)
    ones128 = np.ones((128, 1), dtype=np.float32)

    wmask_all = (np.arange(LW)[None, :] < s_l[:, None]).astype(np.float32)  # [B, LW]
    capn_all = np.linalg.norm(s, axis=-1)                                    # [B, LW]

    in_maps = []
    for core in range(NC):
        j0 = core * CAP
        sj = s[j0 : j0 + CAP]                       # [CAP, LW, D]
        sT = sj.reshape(WF, D).T                    # [D, WF]
        sT8 = np.ascontiguousarray(sT.reshape(8, 128, WF))
        wm = wmask_all[j0 : j0 + CAP]               # [CAP, LW]
        capn = capn_all[j0 : j0 + CAP]
        maskneg = ((1.0 - wm) * MASKNEG).reshape(1, WF).astype(np.float32)
        lens = s_l[j0 : j0 + CAP].astype(np.float32)[:, None]
        wfac = (wm / (np.maximum(capn, EPS) * lens)).reshape(WF).astype(np.float32)
        wfac = np.broadcast_to(wfac, (128, WF)).copy()
        in_maps.append(
            {
                "imT8": imT8,
                "sT8": sT8,
                "g43": g43,
                "maskneg": maskneg,
                "wfac": wfac,
                "eye": eye,
                "noteye": noteye,
                "onesblk": onesblk,
                "ones1": ones1,
                "ones128": ones128,
            }
        )
    return in_maps


def run(im, s, s_l, trace=False):
    """Returns (loss_scalar, scores[128,128], bass_results)."""
    _install_patches()
    if "nc" not in _CACHE:
        _CACHE["nc"] = _build_program()
    nc = _CACHE["nc"]
    in_maps = _host_prep(im, s, s_l)
    try:
        res = run_bass_kernel_spmd(nc, in_maps, list(range(NC)), trace=trace)
    except ModuleNotFoundError:
        # NTFF profile hook unavailable in this image; run without tracing.
        res = run_bass_kernel_spmd(nc, in_maps, list(range(NC)), trace=False)
    r0 = res.results[0]
    loss = np.float32(r0["loss_out"][0, 0] + r0["loss_out"][0, 1])
    return loss, r0["scores_out"], res


def kernel(im, s, s_l):
    loss, _, _ = run(im, s, s_l)
    return np.array(loss, dtype=np.float32)
